# revision 7
# baseline (speedup 1.0000x reference)
"""Trainium2 Bass kernel for nn_CTCConsistencyLoss_7310034338203 (v5).

v5: meet-in-the-middle. The forward (alpha) chain runs t=1..TMEET while an
independent backward (suffix) chain runs t=TMAX-2..TMEET concurrently on the
same engines in antiphase — each chain's serial latency (~560ns/step) hides
the other's, halving the dominant DP wall time. The backward recurrence
  delta_t[s] = U[s] + E1*U[s+1] + E2*skip[s+2]*U[s+2],  U = delta_{t+1} o P_{t+1}
is tilted as delta~[s] = e^{-CTILT*s} * e^{CTILT*2tl} * delta[s] so its shift
weights are the SAME bf16-exact E1/E2 and the meet
  P_b * e^{C1*il} * e^{CTILT*2tl} = sum_s alphatilde_TMEET[s] * deltatilde_TMEET[s]
needs only the existing C1IL correction. Per-sample end states inject via
rank-4 per-t accumulating matmuls (host one-hots); samples with il == TMAX
seed the backward init tile. Both chains rescale every 128 steps with gated
log-sum storage (single batched Ln at readout).
"""
import math

import numpy as np
import ml_dtypes

B, T, D, V, L = 32, 1000, 768, 31, 200
S = 2 * L + 1
NJ = 7                       # chunks of 64 states, no halo
BLK = 64
B_LOC = 4
NCOL = B_LOC * NJ            # 28
C1 = 3.25                    # per-frame boost: keeps X log-drift near 0
E1 = 13.0 / 32.0             # bf16-exact tilt weight
E2 = E1 * E1                 # 169/1024, bf16-exact
CTILT = math.log(E1)
RESC = 128
BF16 = ml_dtypes.bfloat16


DS = 64                      # forward solo steps before backward starts


def _plan(tmax, inj_lo):
    tmeet = (tmax - 1 + DS) // 2      # balance: fw starts DS supersteps early
    nbw = tmax - 1 - tmeet            # backward iterations
    ntb = (tmax - 1) - inj_lo         # injection slots for t* in [inj_lo, tmax-1)
    fw_resc = [k for k in range(RESC, tmeet + 1, RESC)]
    bw_resc = [i for i in range(RESC, nbw + 1, RESC)]
    # gated-log t_k values: fw t_k = t; bw t_k = (tmax-1) - i + 1
    tks = fw_resc + [tmax - i for i in bw_resc]
    return tmeet, nbw, ntb, fw_resc, bw_resc, tks


def _build_core_consts(tgt2d, in_len, tgt_len, b_base, tmax, inj_lo):
    il = in_len[b_base:b_base + B_LOC].astype(np.int64)
    tl = tgt_len[b_base:b_base + B_LOC].astype(np.int64)
    tmeet, nbw, ntb, fw_resc, bw_resc, tks = _plan(tmax, inj_lo)
    ext = np.zeros((B_LOC, S), np.int64)
    ext[:, 1::2] = tgt2d[b_base:b_base + B_LOC]
    skip = np.zeros((B_LOC, S), bool)
    skip[:, 2:] = (ext[:, 2:] != 0) & (ext[:, 2:] != ext[:, :-2])
    m2s = np.zeros((B_LOC, S + 2), bool)
    m2s[:, :S - 2] = skip[:, 2:]

    G = np.zeros((V, NCOL, 128), np.float32)
    for b in range(B_LOC):
        for j in range(NJ):
            for m in range(BLK):
                s = BLK * j + m
                if s <= 2 * int(tl[b]):
                    G[ext[b, s], b * NJ + j, m] = 1.0
                    if m2s[b, s]:
                        G[ext[b, s], b * NJ + j, 64 + m] = 1.0
    g_in = G.reshape(V, NCOL * 128).astype(BF16)

    W1 = np.zeros((128, 128), np.float32)
    for m in range(BLK):
        W1[m, m] = 1.0
        if m >= 1:
            W1[m - 1, m] = E1
        if m >= 2:
            W1[64 + m - 2, m] = E2
        W1[:, 64 + m] = W1[:, m]
    PK = np.zeros((128, 128), np.float32)
    PK[63, 0] = E1
    PK[126, 0] = E2
    PK[127, 1] = E2
    PK[:, 64] = PK[:, 0]
    PK[:, 65] = PK[:, 1]

    # backward: delta[m] = U[m] + E1*U[m+1] + E2*Ubeta[m+2]; rows 64:128 of
    # the backward state replicate rows 0:64 (so the next mul produces both
    # the plain and the skip-masked products from PQ's alpha/beta rows)
    WB = np.zeros((128, 128), np.float32)
    for m in range(BLK):
        WB[m, m] = 1.0
        if m + 1 < BLK:
            WB[m + 1, m] = E1
        if m + 2 < BLK:
            WB[64 + m + 2, m] = E2
        WB[:, 64 + m] = WB[:, m]
    PKB = np.zeros((128, 128), np.float32)
    PKB[64, 62] = E2          # s=64j+62: s+2 = chunk j+1 beta row 0
    PKB[0, 63] = E1           # s=64j+63: s+1 = chunk j+1 row 0
    PKB[65, 63] = E2          # s=64j+63: s+2 = chunk j+1 beta row 1
    PKB[:, 64 + 62] = PKB[:, 62]
    PKB[:, 64 + 63] = PKB[:, 63]

    OH01 = np.zeros((128, NCOL), np.float32)
    for b in range(B_LOC):
        c = b * NJ
        OH01[0, c] = 1.0
        OH01[1, c] = E1
        OH01[64, c] = 1.0
        OH01[65, c] = E1
    CM = np.zeros((128, 1), np.float32)
    CM[0:64, 0] = 1.0

    # backward init (t* == tmax-1) and per-t injections (t* < tmax-1)
    XB0 = np.zeros((128, NCOL), np.float32)
    INJL = np.zeros((8, ntb, 128), np.float32)
    INJR = np.zeros((8, ntb, NCOL), np.float32)
    slot_used = np.zeros(ntb, np.int64)
    for b in range(B_LOC):
        tstar = int(il[b]) - 1
        for i, sstar in enumerate([2 * int(tl[b]) - 1, 2 * int(tl[b])]):
            jstar = sstar // BLK
            p = sstar - BLK * jstar
            w = E1 if i == 0 else 1.0
            col = b * NJ + jstar
            if tstar == tmax - 1:
                XB0[p, col] += w
                XB0[64 + p, col] += w
            else:
                ti = tstar - inj_lo
                q = slot_used[ti]
                INJL[q, ti, p] = 1.0
                INJL[q, ti, 64 + p] = 1.0
                INJR[q, ti, col] = w
                slot_used[ti] += 1

    C1IL = (C1 * il + CTILT * 2 * tl).astype(np.float32).reshape(B_LOC, 1)
    RTL = (1.0 / tl.astype(np.float64)).astype(np.float32).reshape(B_LOC, 1)
    nk = len(tks)
    M0 = np.zeros((1, nk * B_LOC), np.float32)
    for k, tk in enumerate(tks):
        M0[0, k * B_LOC:(k + 1) * B_LOC] = (tk < il).astype(np.float32)
    M1 = 1.0 - M0
    ID = np.eye(125, dtype=np.float32)
    EC = np.full((1, 128), math.exp(C1), np.float32)
    return dict(g=g_in, w1=W1.astype(BF16), pk=PK.astype(BF16),
                wb=WB.astype(BF16), pkb=PKB.astype(BF16),
                oh01=OH01.astype(BF16), cmask=CM.astype(BF16),
                xb0=XB0.astype(BF16),
                injl=INJL.reshape(8, ntb * 128).astype(BF16),
                injr=INJR.reshape(8, ntb * NCOL).astype(BF16),
                c1il=C1IL, rtl=RTL, mask0=M0, mask1=M1,
                one1=np.ones((1, 1), np.float32),
                id125=ID, ecrow=EC)


def build_program(stage=5, tmax=992, inj_lo=805):
    import concourse.bacc as bacc
    import concourse.tile as tile
    from concourse import mybir

    f32 = mybir.dt.float32
    bf16 = mybir.dt.bfloat16
    AF = mybir.ActivationFunctionType
    ALU = mybir.AluOpType
    AX = mybir.AxisListType
    tmeet, nbw, ntb, fw_resc, bw_resc, tks = _plan(tmax, inj_lo)
    nk = len(tks)

    nc = bacc.Bacc("TRN2", target_bir_lowering=False, debug=False,
                   enable_asserts=False)

    pred = nc.dram_tensor("pred", [B_LOC, T, D], f32, kind="ExternalInput").ap()
    wt = nc.dram_tensor("wt", [D, V], bf16, kind="ExternalInput").ap()
    bb = nc.dram_tensor("bb", [V, 1], f32, kind="ExternalInput").ap()
    g_d = nc.dram_tensor("g", [V, NCOL * 128], bf16, kind="ExternalInput").ap()
    w1_d = nc.dram_tensor("w1", [128, 128], bf16, kind="ExternalInput").ap()
    pk_d = nc.dram_tensor("pk", [128, 128], bf16, kind="ExternalInput").ap()
    wb_d = nc.dram_tensor("wb", [128, 128], bf16, kind="ExternalInput").ap()
    pkb_d = nc.dram_tensor("pkb", [128, 128], bf16, kind="ExternalInput").ap()
    oh01_d = nc.dram_tensor("oh01", [128, NCOL], bf16, kind="ExternalInput").ap()
    cm_d = nc.dram_tensor("cmask", [128, 1], bf16, kind="ExternalInput").ap()
    xb0_d = nc.dram_tensor("xb0", [128, NCOL], bf16, kind="ExternalInput").ap()
    injl_d = nc.dram_tensor("injl", [8, ntb * 128], bf16, kind="ExternalInput").ap()
    injr_d = nc.dram_tensor("injr", [8, ntb * NCOL], bf16, kind="ExternalInput").ap()
    c1il_d = nc.dram_tensor("c1il", [B_LOC, 1], f32, kind="ExternalInput").ap()
    rtl_d = nc.dram_tensor("rtl", [B_LOC, 1], f32, kind="ExternalInput").ap()
    m0_d = nc.dram_tensor("mask0", [1, nk * B_LOC], f32, kind="ExternalInput").ap()
    m1_d = nc.dram_tensor("mask1", [1, nk * B_LOC], f32, kind="ExternalInput").ap()
    one1_d = nc.dram_tensor("one1", [1, 1], f32, kind="ExternalInput").ap()
    id_d = nc.dram_tensor("id125", [125, 125], f32, kind="ExternalInput").ap()
    ec_d = nc.dram_tensor("ecrow", [1, 128], f32, kind="ExternalInput").ap()
    y_d = nc.dram_tensor("y", [B_LOC, 1], f32, kind="ExternalOutput").ap()

    dma = nc.default_dma_engine.dma_start
    dma2 = nc.scalar.dma_start

    with tile.TileContext(nc) as tc:
        with (
            tc.tile_pool(name="consts", bufs=1) as cp,
            tc.tile_pool(name="persist", bufs=1) as pp,
            tc.tile_pool(name="xs", bufs=3) as xp,
            tc.tile_pool(name="ubs", bufs=3) as ubp,
            tc.tile_pool(name="small", bufs=2) as smp,
            tc.tile_pool(name="pnat", bufs=4) as pnp,
            tc.tile_pool(name="predT", bufs=2) as ptp,
            tc.tile_pool(name="epool", bufs=2) as ep,
            tc.tile_pool(name="enpool", bufs=8) as enp,
            tc.tile_pool(name="rcpool", bufs=2) as rcpool,
        ):
            pn0 = {}
            for h in range(2):
                for b in range(B_LOC):
                    pnt = pnp.tile([125, 4, D], f32, tag="pn",
                                   name=f"pn_{b}_{h}")
                    dma(pnt, pred[b, h * 500:(h + 1) * 500, :]
                        .rearrange("(k p) d -> p k d", p=125))
                    pn0[(b, h)] = pnt
            id_sb = cp.tile([125, 125], f32, tag="id125")
            dma2(id_sb, id_d)
            wt_sb = cp.tile([128, 6, V], bf16, tag="wt")
            dma2(wt_sb, wt.rearrange("(c p) v -> p c v", p=128))
            bb_sb = cp.tile([V, 1], f32, tag="bb")
            dma2(bb_sb, bb)
            g_sb = cp.tile([V, NCOL, 128], bf16, tag="g")
            dma2(g_sb, g_d.rearrange("v (c m) -> v c m", m=128))
            ec_sb = cp.tile([1, 128], f32, tag="ecrow")
            dma2(ec_sb, ec_d)
            w1_sb = cp.tile([128, 128], bf16, tag="w1")
            dma2(w1_sb, w1_d)
            pk_sb = cp.tile([128, 128], bf16, tag="pk")
            dma2(pk_sb, pk_d)
            wb_sb = cp.tile([128, 128], bf16, tag="wb")
            dma2(wb_sb, wb_d)
            pkb_sb = cp.tile([128, 128], bf16, tag="pkb")
            dma2(pkb_sb, pkb_d)
            oh01_sb = cp.tile([128, NCOL], bf16, tag="oh01")
            dma2(oh01_sb, oh01_d)
            cm_sb = cp.tile([128, 1], bf16, tag="cmask")
            dma2(cm_sb, cm_d)
            xb0_sb = cp.tile([128, NCOL], bf16, tag="xb0")
            dma2(xb0_sb, xb0_d)
            injl_sb = cp.tile([8, ntb, 128], bf16, tag="injl")
            dma2(injl_sb, injl_d.rearrange("p (t c) -> p t c", c=128))
            injr_sb = cp.tile([8, ntb, NCOL], bf16, tag="injr")
            dma2(injr_sb, injr_d.rearrange("p (t c) -> p t c", c=NCOL))
            c1il_sb = cp.tile([B_LOC, 1], f32, tag="c1il")
            dma2(c1il_sb, c1il_d)
            rtl_sb = cp.tile([B_LOC, 1], f32, tag="rtl")
            dma2(rtl_sb, rtl_d)
            m0_sb = cp.tile([1, nk * B_LOC], f32, tag="mask0")
            dma2(m0_sb, m0_d)
            m1_sb = cp.tile([1, nk * B_LOC], f32, tag="mask1")
            dma2(m1_sb, m1_d)
            one1_sb = cp.tile([1, 1], f32, tag="one1")
            dma2(one1_sb, one1_d)
            ones_col = cp.tile([128, 1], bf16, tag="ones_col")
            nc.vector.memset(ones_col, 1.0)
            ones_row = cp.tile([1, 128], f32, tag="ones_row")
            nc.vector.memset(ones_row, 1.0)

            PQ = pp.tile([128, NCOL, T], bf16, tag="PQ")
            SLOG = pp.tile([1, nk * B_LOC], f32, tag="SLOG")

            cnt = [0]
            ens = {}

            def prep_units(b, h, pools, act_only=False):
                ptrp, plogp, pzbp, _ = pools
                st = {"pn": pn0[(b, h)]}

                def mk_trans_pair(c, half):
                    def u_trans():
                        if half == 0:
                            st["ptr"] = ptrp.tile([128, 500], f32, tag="ptr",
                                                  name="ptr")
                        for k in (0, 1) if half == 0 else (2, 3):
                            nc.tensor.matmul(
                                st["ptr"][:, k * 125:(k + 1) * 125],
                                st["pn"][:, k, c * 128:(c + 1) * 128],
                                id_sb, is_transpose=True,
                                start=True, stop=True)
                        if half == 1:
                            cnt[0] += 1
                            if not act_only and cnt[0] % 2 == 0:
                                nc.vector.tensor_copy(st["pt"][:, c, :],
                                                      st["ptr"])
                            else:
                                nc.scalar.copy(st["pt"][:, c, :], st["ptr"])
                    return u_trans

                def u_start():
                    st["pt"] = ptp.tile([128, 6, 500], bf16, tag="pt",
                                        name="pt")
                yield u_start
                for c in range(6):
                    yield mk_trans_pair(c, 0)
                    yield mk_trans_pair(c, 1)

                def mk_wmm(c):
                    def u_wmm():
                        if c == 0:
                            st["pslog"] = plogp.tile([V, 500], f32,
                                                     tag="pslog", name="pslog")
                        nc.tensor.matmul(st["pslog"], wt_sb[:, c, :],
                                         st["pt"][:, c, :],
                                         start=(c == 0), stop=(c == 5),
                                         skip_group_check=True)
                    return u_wmm
                for c in range(6):
                    yield mk_wmm(c)

                def u_exp():
                    Es = ep.tile([V, 500], bf16, tag="Es")
                    nc.scalar.activation(Es, st["pslog"], AF.Exp, bias=bb_sb)
                    st["Es"] = Es
                yield u_exp

                def u_norm():
                    pzb = pzbp.tile([128, 500], f32, tag="pzb")
                    nc.tensor.matmul(pzb[32:33, :], ones_col[0:V, :],
                                     st["Es"], start=True, stop=True)
                    rc = rcpool.tile([1, 500], f32, tag="rc")
                    nc.vector.reciprocal(rc, pzb[32:33, :])
                    nc.tensor.matmul(pzb[0:V, :], ec_sb[:, 0:V], rc,
                                     start=True, stop=True)
                    En = enp.tile([V, 500], bf16, tag="En")
                    nc.vector.tensor_mul(En, st["Es"], pzb[0:V, :])
                    ens[(b, h)] = En
                yield u_norm

            def mk_gather(b, h, j, tlo, thi, pgp, act_only):
                def u_gather():
                    gidx = b * NJ + j
                    pg = pgp.tile([128, 500], f32, tag="psg")
                    nc.tensor.matmul(pg[:, 0:thi - tlo], g_sb[:, gidx, :],
                                     ens[(b, h)][:, tlo:thi],
                                     start=True, stop=True)
                    dst = PQ[:, gidx, h * 500 + tlo:h * 500 + thi]
                    cnt[0] += 1
                    if not act_only and cnt[0] % 2 == 0:
                        nc.vector.tensor_copy(dst, pg[:, 0:thi - tlo])
                    else:
                        nc.scalar.copy(dst, pg[:, 0:thi - tlo])
                return u_gather

            # ---- all 8 preps with fat pools ---------------------------------
            with (
                tc.tile_pool(name="ptrA", bufs=2, space="PSUM") as ptrA,
                tc.tile_pool(name="plogA", bufs=2, space="PSUM") as plogA,
                tc.tile_pool(name="pzbA", bufs=1, space="PSUM") as pzbA,
                tc.tile_pool(name="pgatA", bufs=2, space="PSUM") as pgA,
            ):
                fat = (ptrA, plogA, pzbA, pgA)
                for b in range(B_LOC):
                    for u in prep_units(b, 0, fat):
                        u()
                # phase A fw: head [0,125)
                for b in range(B_LOC):
                    for j in range(NJ):
                        mk_gather(b, 0, j, 0, 125, pgA, False)()

            # ---- chains + phase-B gathers -----------------------------------
            with (
                tc.tile_pool(name="ptrB", bufs=1, space="PSUM") as ptrB,
                tc.tile_pool(name="plogB", bufs=1, space="PSUM") as plogB,
                tc.tile_pool(name="pzbB", bufs=1, space="PSUM") as pzbB,
                tc.tile_pool(name="pgB", bufs=1, space="PSUM") as pgB,
                tc.tile_pool(name="pfw", bufs=1, space="PSUM") as pfwp,
                tc.tile_pool(name="pbw", bufs=2, space="PSUM") as pbwp,
                tc.tile_pool(name="presc", bufs=1, space="PSUM") as prescp,
            ):
                thin = (ptrB, plogB, pzbB, pgB)
                units = []
                for b in range(B_LOC):
                    units += list(prep_units(b, 1, thin, act_only=True))
                for b in range(B_LOC):
                    for j in range(NJ):
                        units.append(mk_gather(b, 1, j, 367, tmax - 500,
                                               pgB, True))
                for b in range(B_LOC):
                    for j in range(NJ):
                        units.append(mk_gather(b, 0, j, 125, 500, pgB, True))
                for b in range(B_LOC):
                    for j in range(NJ):
                        units.append(mk_gather(b, 1, j, 0, 367, pgB, True))

                Xf = xp.tile([128, NCOL], bf16, tag="Xf")
                nc.vector.tensor_mul(Xf, PQ[:, :, 0], oh01_sb)
                psb = xb0_sb               # backward "PSUM" for i=1 is SBUF
                ui = 0
                rk = [0]

                def rescale(Xin, pool_tag, tk_idx):
                    rt = prescp.tile([128, 32], f32, tag="resc",
                                     name=f"rt{tk_idx}")
                    nc.tensor.matmul(rt[0:1, 0:NCOL], cm_sb, Xin,
                                     start=True, stop=True)
                    s4 = smp.tile([1, B_LOC], f32, tag="s4")
                    nc.vector.tensor_reduce(
                        s4, rt[0:1, 0:NCOL].rearrange("p (b j) -> p b j", j=NJ),
                        axis=AX.X, op=ALU.add)
                    # gated sum s' = s*m + (1-m): masked samples rescale by
                    # recip(1) = 1 (their columns are still all-zero)
                    sg = smp.tile([1, B_LOC], f32, tag="sg")
                    nc.vector.tensor_mul(
                        sg, s4, m0_sb[:, tk_idx * B_LOC:(tk_idx + 1) * B_LOC])
                    nc.vector.tensor_add(
                        SLOG[:, tk_idx * B_LOC:(tk_idx + 1) * B_LOC], sg,
                        m1_sb[:, tk_idx * B_LOC:(tk_idx + 1) * B_LOC])
                    r4 = smp.tile([1, B_LOC], f32, tag="r4")
                    nc.vector.reciprocal(
                        r4, SLOG[:, tk_idx * B_LOC:(tk_idx + 1) * B_LOC])
                    nc.tensor.matmul(rt[:, 28:32], ones_row, r4,
                                     start=True, stop=True)
                    Xr = (xp if pool_tag == "Xf" else ubp).tile(
                        [128, NCOL], bf16, tag=pool_tag, name=f"xr{tk_idx}")
                    import concourse.bass as _bass
                    xv3 = Xin.rearrange("p (b j) -> p b j", j=NJ)
                    rt3 = rt[:, 28:32].rearrange("p (c u) -> p c u", u=1)
                    _, rt3b = _bass.broadcast_tensor_aps(xv3, rt3)
                    nc.vector.tensor_mul(
                        Xr.rearrange("p (b j) -> p b j", j=NJ), xv3, rt3b)
                    return Xr

                niter = max(tmeet, DS + nbw)
                for i in range(1, niter + 1):
                    nu = 2 if i <= DS else (1 if i % 2 == 0 else 0)
                    for _ in range(nu):
                        if ui < len(units):
                            units[ui]()
                            ui += 1
                    # ---------------- forward step i (t = i) ---------------
                    if i <= tmeet:
                        if i in fw_resc:
                            Xf = rescale(Xf, "Xf", fw_resc.index(i))
                        ps = pfwp.tile([128, NCOL], f32, tag="ps")
                        psv = ps.rearrange("p (b j) -> p b j", j=NJ)
                        xvp = Xf.rearrange("p (b j) -> p b j", j=NJ)
                        nc.tensor.matmul(ps, w1_sb, Xf, start=True, stop=False,
                                         skip_group_check=True)
                        nc.tensor.matmul(psv[:, :, 1:NJ], pk_sb,
                                         xvp[:, :, 0:NJ - 1],
                                         start=False, stop=True,
                                         skip_group_check=True)
                        Xf = xp.tile([128, NCOL], bf16, tag="Xf")
                        nc.vector.tensor_mul(Xf, ps, PQ[:, :, i])
                    # ------------- backward step k = i-DS ------------------
                    if DS < i <= DS + nbw:
                        k = i - DS
                        tb = tmax - 1 - k
                        Ub = ubp.tile([128, NCOL], bf16, tag="Ub")
                        nc.vector.tensor_mul(Ub, psb, PQ[:, :, tb + 1])
                        if k in bw_resc:
                            Ub = rescale(Ub, "Ub",
                                         len(fw_resc) + bw_resc.index(k))
                        pb = pbwp.tile([128, NCOL], f32, tag="pb")
                        pbv = pb.rearrange("p (b j) -> p b j", j=NJ)
                        uvp = Ub.rearrange("p (b j) -> p b j", j=NJ)
                        has_inj = inj_lo <= tb < tmax - 1
                        nc.tensor.matmul(pb, wb_sb, Ub, start=True, stop=False,
                                         skip_group_check=True)
                        nc.tensor.matmul(pbv[:, :, 0:NJ - 1], pkb_sb,
                                         uvp[:, :, 1:NJ],
                                         start=False, stop=not has_inj,
                                         skip_group_check=True)
                        if has_inj:
                            ti = tb - inj_lo
                            nc.tensor.matmul(pb, injl_sb[:, ti, :],
                                             injr_sb[:, ti, :],
                                             start=False, stop=True,
                                             skip_group_check=True)
                        psb = pb

                # ---------------- meet + readout -------------------------
                if stage >= 5:
                    mt = smp.tile([64, NCOL], bf16, tag="mt")
                    nc.vector.tensor_mul(mt, psb[0:64, :], Xf[0:64, :])
                    rtm = prescp.tile([128, 32], f32, tag="resc", name="rtm")
                    nc.tensor.matmul(rtm[0:1, 0:NCOL], cm_sb[0:64, :], mt,
                                     start=True, stop=True)
                    v4 = smp.tile([1, B_LOC], f32, tag="v4")
                    nc.vector.tensor_reduce(
                        v4, rtm[0:1, 0:NCOL].rearrange("p (b j) -> p b j", j=NJ),
                        axis=AX.X, op=ALU.add)
                    lnv = smp.tile([1, B_LOC], f32, tag="lnv")
                    nc.scalar.activation(lnv, v4, AF.Ln)
                    lsl = smp.tile([1, nk * B_LOC], f32, tag="lsl")
                    nc.scalar.activation(lsl, SLOG, AF.Ln)
                    red4 = smp.tile([1, B_LOC], f32, tag="red4")
                    nc.vector.tensor_reduce(
                        red4, lsl.rearrange("p (k b) -> p b k", b=B_LOC),
                        axis=AX.X, op=ALU.add)
                    accr = smp.tile([1, B_LOC], f32, tag="accr")
                    nc.vector.tensor_add(accr, red4, lnv)
                    rta = prescp.tile([128, 32], f32, tag="resc", name="rta")
                    ps_a4 = rta[0:B_LOC, 0:1]
                    nc.tensor.matmul(ps_a4, accr, one1_sb, start=True, stop=True)
                    t1 = smp.tile([B_LOC, 1], f32, tag="t1")
                    nc.vector.tensor_sub(t1, c1il_sb, ps_a4)
                    t3 = smp.tile([B_LOC, 1], f32, tag="t3")
                    nc.vector.tensor_mul(t3, t1, rtl_sb)
                    dma(y_d, t3)
                else:
                    dbg = smp.tile([B_LOC, 1], f32, tag="dbg")
                    nc.vector.tensor_copy(dbg, Xf[0:B_LOC, 0:1])
                    dma(y_d, dbg)

    nc.compile()
    return nc


def build_in_maps(inputs):
    pred = np.ascontiguousarray(np.asarray(inputs["pred"], np.float32))
    targets = np.asarray(inputs["targets"]).astype(np.int64)
    in_len = np.asarray(inputs["input_lengths"]).astype(np.int64)
    tgt_len = np.asarray(inputs["target_lengths"]).astype(np.int64)
    Wm = np.asarray(inputs["W"], np.float32)
    bv = np.asarray(inputs["b"], np.float32)
    tgt2d = targets.reshape(B, L)
    wt = np.ascontiguousarray(Wm.T.astype(BF16))
    bb = np.ascontiguousarray(bv.reshape(V, 1))
    tmax = int(in_len.max())
    inj_lo = int(in_len.min()) - 1
    in_maps = []
    for core in range(8):
        b0 = core * B_LOC
        cst = _build_core_consts(tgt2d, in_len, tgt_len, b0, tmax, inj_lo)
        im = dict(pred=np.ascontiguousarray(pred[b0:b0 + B_LOC]),
                  wt=wt, bb=bb)
        for k, v in cst.items():
            im[k] = np.ascontiguousarray(v)
        in_maps.append(im)
    return in_maps


_CACHED = {}


def kernel(**inputs):
    from concourse import bass_utils
    il = np.asarray(inputs["input_lengths"]).astype(np.int64)
    key = (int(il.max()), int(il.min()) - 1)
    if _CACHED.get("key") != key:
        _CACHED["nc"] = build_program(tmax=key[0], inj_lo=key[1])
        _CACHED["key"] = key
    nc = _CACHED["nc"]
    in_maps = build_in_maps(inputs)
    res = bass_utils.run_bass_kernel_spmd(nc, in_maps, core_ids=list(range(8)))
    ys = [r["y"] for r in res.results]
    loss = np.concatenate([y.ravel() for y in ys]).astype(np.float64).sum() / B
    return np.float32(loss)
